# revision 2
# baseline (speedup 1.0000x reference)
"""Trainium2 Bass kernel for the 21-joint hand-graph message-passing MLP.

Math (per sample b, per target joint t with neighbor list S_t of length n):
    g   = concat(x[b, S_t[0]], ..., x[b, S_t[n-1]])          # [n*64]
    h1  = relu(g @ W1_t + b1_t)                              # [128]
    h2  = relu(h1 @ W2_t + b2_t)                             # [128]
    out[b, t] = h2 @ W3_t + b3_t                             # [64]

Strategy (pure data parallel over 8 NeuronCores, B=65536 -> 8192/core):
  - x is pre-transposed ON THE HOST into a feature-major pair-tile layout
    xpk [15*128, BC] bf16: block p holds the 64 features of node u_p on
    partitions 0-63 and node v_p on 64-127.  The kernel reads it with ONE
    plain 3.84MB DMA per 1024-batch tile (sync HWDGE ring) -- no xbar
    transposes at all.  Host-side pair packing makes ARBITRARY node pairs
    available, so L1 runs the optimal per-target pairing: 28 genuine
    K=128 pair chunks + 13 zero-padded singles = 41 chunks (vs 47 for
    the adjacent-pair tile set), all full 128-row matmuls (partial-row
    matmuls that mix row groups in a PSUM bank can fault the device).
  - L2 is weight-stationary (w2 [128,128]); relu+bias fused into the PSUM
    evacuations, greedily load-balanced between ScalarE and VectorE.
  - L3 is w3-stationary with h2 moving at N=512: two adjacent targets
    share one 2-bank PSUM tile in disjoint col groups (concurrent
    matmuls); b3 is a per-partition bias folded into the single [128,1024]
    evacuation, which writes bf16.  Stores go out on the gpsimd SWDGE
    queues (separate from the load ring).  The feature-major [1344, BC]
    bf16 result is transposed back to [B, 21, 64] f32 on the host.
  - The PE stream is software-pipelined (L1(k) | L2(k-2) | L3(k-4)),
    continuous across batch-tile boundaries; the 2-step L2 lag hides the
    h1 PSUM-evacuation latency behind a full step of L1 matmuls.
"""

import numpy as np
import ml_dtypes

B, J, D, H1, H2 = 65536, 21, 64, 128, 128
NCORES = 8
BC = B // NCORES          # 8192 samples per core
TILE = 1024               # batch tile (psum1/psum2 = 2 PSUM banks in fp32)
NTILES = BC // TILE       # 8
TPAIRS = 11               # L3 target pairs: (0,1),(2,3),...,(18,19),(20,)

# Host-packed x pair tiles: ANY (u, v) node pair can be a tile because the
# host lays them out adjacently.  Chosen so every target's neighbor list
# splits into ceil(n/2) chunks (28 pairs + 13 singles = 41 chunks).
XTILES = [(0, 13), (1, 5), (9, 17), (0, 5), (1, 2), (3, 4), (5, 6),
          (0, 1), (7, 8), (9, 10), (11, 12), (13, 14), (15, 16),
          (17, 18), (19, 20)]
NPAIRS = len(XTILES)
PAIR_TILE = {p: i for i, p in enumerate(XTILES)}
TILE_OF = {}              # node -> (tile_idx, half), first occurrence
for _i, (_u, _v) in enumerate(XTILES):
    TILE_OF.setdefault(_u, (_i, 0))
    TILE_OF.setdefault(_v, (_i, 1))

FINGER_BASE = [4 * f + 1 for f in range(5)]
NEIGH = {
    6: [[0, 1, 5, 9, 13, 17]],
    5: [[0, 5, 6, 1, 9], [0, 9, 10, 5, 13], [0, 13, 14, 9, 17]],
    4: [[0, 1, 2, 5], [0, 17, 18, 13]],
    3: [r for b in FINGER_BASE for r in ([b, b + 1, b + 2], [b + 1, b + 2, b + 3])],
    2: [[b + 2, b + 3] for b in FINGER_BASE],
}
OUT = {
    6: [0],
    5: [5, 9, 13],
    4: [1, 17],
    3: [j for b in FINGER_BASE for j in (b + 1, b + 2)],
    2: [b + 3 for b in FINGER_BASE],
}
GROUPS = [6, 5, 4, 3, 2]

# target t -> (group n, row index within its group, neighbor list)
TARGET = {}
for n in GROUPS:
    for row, t in enumerate(OUT[n]):
        TARGET[t] = (n, row, list(NEIGH[n][row]))

# Hand-chosen optimal pairing per target (node pairs; all are in XTILES).
PAIRING = {
    0: [(0, 13), (1, 5), (9, 17)],
    1: [(0, 5), (1, 2)],
    2: [(1, 2)],
    3: [(3, 4)],
    4: [(3, 4)],
    5: [(5, 6), (0, 1)],
    6: [(5, 6)],
    7: [(7, 8)],
    8: [(7, 8)],
    9: [(9, 10), (0, 13)],
    10: [(9, 10)],
    11: [(11, 12)],
    12: [(11, 12)],
    13: [(13, 14), (9, 17)],
    14: [(13, 14)],
    15: [(15, 16)],
    16: [(15, 16)],
    17: [(17, 18), (0, 13)],
    18: [(17, 18)],
    19: [(19, 20)],
    20: [(19, 20)],
}


def build_l1_plan():
    plan = {}
    for t in range(21):
        n, _, S = TARGET[t]
        used = [False] * n
        pairs = []
        for (u, v) in PAIRING[t]:
            i, k = S.index(u), S.index(v)
            assert not used[i] and not used[k]
            used[i] = used[k] = True
            pairs.append(dict(tile=PAIR_TILE[(u, v)], pos0=i, pos1=k))
        singles = []
        for i in range(n):
            if not used[i]:
                tile_idx, half = TILE_OF[S[i]]
                singles.append(dict(tile=tile_idx, pos=i, half=half))
        assert len(singles) <= 1
        plan[t] = dict(pairs=pairs, singles=singles)
    return plan


L1_PLAN = build_l1_plan()


def assign_w1_cols():
    cols = {}
    col = 0
    for t in range(21):
        p = L1_PLAN[t]
        for i, _ in enumerate(p["pairs"]):
            cols[(t, "pair", i)] = col
            col += 128
        for i, _ in enumerate(p["singles"]):
            cols[(t, "single", i)] = col
            col += 128
    return cols, col


W1_COLS, W1_NCOL = assign_w1_cols()      # 41 * 128 = 5248


def pack_weights(inputs):
    """Host-side prep: permute/pack all weights into flat bf16/f32 arrays."""
    bf16 = ml_dtypes.bfloat16
    w1p = np.zeros((128, W1_NCOL), np.float32)
    for t in range(21):
        n, row, S = TARGET[t]
        W1 = np.asarray(inputs[f"w1_g{n}"][row], np.float32)  # [n*64, 128]
        p = L1_PLAN[t]
        for i, pr in enumerate(p["pairs"]):
            c = W1_COLS[(t, "pair", i)]
            w1p[0:64, c:c + 128] = W1[64 * pr["pos0"]:64 * pr["pos0"] + 64]
            w1p[64:128, c:c + 128] = W1[64 * pr["pos1"]:64 * pr["pos1"] + 64]
        for i, e in enumerate(p["singles"]):
            c = W1_COLS[(t, "single", i)]
            half = e["half"]
            w1p[64 * half:64 * half + 64, c:c + 128] = \
                W1[64 * e["pos"]:64 * e["pos"] + 64]
    w2p = np.zeros((128, 128 * 21), np.float32)
    w3p = np.zeros((128, 64 * 21), np.float32)
    b1p = np.zeros((128, 21), np.float32)
    b2p = np.zeros((128, 21), np.float32)
    b3p = np.zeros((128, TPAIRS), np.float32)   # per-partition bias, paired
    for t in range(21):
        n, row, _ = TARGET[t]
        w2p[:, 128 * t:128 * (t + 1)] = np.asarray(inputs[f"w2_g{n}"][row])
        w3p[:, 64 * t:64 * (t + 1)] = np.asarray(inputs[f"w3_g{n}"][row])
        b1p[:, t] = np.asarray(inputs[f"b1_g{n}"][row])
        b2p[:, t] = np.asarray(inputs[f"b2_g{n}"][row])
        b3p[64 * (t % 2):64 * (t % 2) + 64, t // 2] = \
            np.asarray(inputs[f"b3_g{n}"][row])
    return dict(
        w1p=w1p.astype(bf16), w2p=w2p.astype(bf16), w3p=w3p.astype(bf16),
        b1p=b1p, b2p=b2p, b3p=b3p,
    )


def pack_x(x):
    """x [Bn, 21, 64] f32 -> xpk [NPAIRS*128, Bn] bf16 (feature-major pairs)."""
    bf16 = ml_dtypes.bfloat16
    Bn = x.shape[0]
    xt = np.ascontiguousarray(
        np.asarray(x, np.float32).astype(bf16).transpose(1, 2, 0))  # [21,64,Bn]
    xpk = np.empty((NPAIRS * 128, Bn), bf16)
    for p, (u, v) in enumerate(XTILES):
        xpk[128 * p:128 * p + 64] = xt[u]
        xpk[128 * p + 64:128 * p + 128] = xt[v]
    return xpk


def numpy_emulate(inputs, x):
    """Bit-layout-faithful numpy model of the HW kernel (minus PSUM rounding):
    validates the chunk plan / packing / L3 pairing offline."""
    bf16 = ml_dtypes.bfloat16
    packed = pack_weights(inputs)
    xpk = pack_x(x)
    Bn = x.shape[0]
    out = np.zeros((Bn, 21, 64), np.float32)
    for t in range(21):
        psum1 = np.zeros((128, Bn), np.float32)
        pl = L1_PLAN[t]
        chunks = [("pair", i, pr["tile"]) for i, pr in enumerate(pl["pairs"])]
        chunks += [("single", i, e["tile"]) for i, e in enumerate(pl["singles"])]
        for kind, i, tl in chunks:
            c = W1_COLS[(t, kind, i)]
            lhsT = packed["w1p"][:, c:c + 128].astype(np.float32)
            rhs = xpk[128 * tl:128 * tl + 128].astype(np.float32)
            psum1 += lhsT.T @ rhs
        h1 = np.maximum(psum1 + packed["b1p"][:, t:t + 1], 0).astype(bf16)
        w2 = packed["w2p"][:, 128 * t:128 * (t + 1)].astype(np.float32)
        psum2 = w2.T @ h1.astype(np.float32)
        h2 = np.maximum(psum2 + packed["b2p"][:, t:t + 1], 0).astype(bf16)
        w3 = packed["w3p"][:, 64 * t:64 * (t + 1)].astype(np.float32)
        b3 = packed["b3p"][64 * (t % 2):64 * (t % 2) + 64, t // 2]
        o = (w3.T @ h2.astype(np.float32) + b3[:, None]).astype(bf16)
        out[:, t] = o.T.astype(np.float32)
    return out


# ---------------------------------------------------------------------------
# Bass kernel
# ---------------------------------------------------------------------------

def build_bass_kernel(num_devices=NCORES, bc=BC):
    import concourse.bass as bass
    import concourse.tile as tile
    from concourse import bacc, mybir

    bf16 = mybir.dt.bfloat16
    f32 = mybir.dt.float32
    Relu = mybir.ActivationFunctionType.Relu
    Ident = mybir.ActivationFunctionType.Identity
    Alu = mybir.AluOpType
    ntiles = bc // TILE

    nc = bacc.Bacc("TRN2", target_bir_lowering=False, debug=False,
                   num_devices=num_devices)
    x_dram = nc.dram_tensor("xpk", [NPAIRS * 128, bc], bf16,
                            kind="ExternalInput").ap()
    out_dram = nc.dram_tensor("outf", [J * D, bc], bf16,
                              kind="ExternalOutput").ap()
    w1_dram = nc.dram_tensor("w1p", [128, W1_NCOL], bf16, kind="ExternalInput").ap()
    w2_dram = nc.dram_tensor("w2p", [128, 128 * 21], bf16, kind="ExternalInput").ap()
    w3_dram = nc.dram_tensor("w3p", [128, 64 * 21], bf16, kind="ExternalInput").ap()
    b1_dram = nc.dram_tensor("b1p", [128, 21], f32, kind="ExternalInput").ap()
    b2_dram = nc.dram_tensor("b2p", [128, 21], f32, kind="ExternalInput").ap()
    b3_dram = nc.dram_tensor("b3p", [128, TPAIRS], f32, kind="ExternalInput").ap()
    # [NPAIRS, 128, bc] view for the one-DMA-per-batch-tile slab load
    x_view = x_dram.rearrange("(p r) b -> r p b", p=NPAIRS, r=128)

    with tile.TileContext(nc) as tc:
        with (
            tc.tile_pool(name="wpool", bufs=1) as wpool,
            tc.tile_pool(name="xtp", bufs=3) as xtp,
            tc.tile_pool(name="h1p", bufs=3) as h1p,
            tc.tile_pool(name="h2p", bufs=1) as h2p,
            tc.tile_pool(name="outp", bufs=6) as outp,
            tc.tile_pool(name="ps12", bufs=3, space="PSUM") as ps12,
            tc.tile_pool(name="ps3", bufs=1, space="PSUM") as ps3,
        ):
            w1s = wpool.tile([128, W1_NCOL], bf16, name="w1s")
            w2s = wpool.tile([128, 128 * 21], bf16, name="w2s")
            w3s = wpool.tile([128, 64 * 21], bf16, name="w3s")
            b1s = wpool.tile([128, 21], f32, name="b1s")
            b2s = wpool.tile([128, 21], f32, name="b2s")
            b3s = wpool.tile([128, TPAIRS], f32, name="b3s")

            # greedy ScalarE/VectorE balance for the PSUM evacuations
            ev_time = [0.0, 0.0]          # ns on [ACT, DVE]

            def evac(dst, src, bias, relu, fd):
                act_cost = (352.0 + fd) / 1.2
                dve_cost = (120.0 + fd) / 0.96
                if ev_time[0] + act_cost <= ev_time[1] + dve_cost:
                    ev_time[0] += act_cost
                    nc.scalar.activation(dst, src, Relu if relu else Ident,
                                         bias=bias, scale=1.0)
                else:
                    ev_time[1] += dve_cost
                    if relu:
                        nc.vector.tensor_scalar(dst, src, bias, 0.0,
                                                Alu.add, Alu.max)
                    else:
                        nc.vector.tensor_scalar(dst, src, bias, None, Alu.add)

            def issue_load(it):
                b0 = it * TILE
                slab = xtp.tile([128, NPAIRS * TILE], bf16, tag="slab",
                                name="slab")
                nc.sync.dma_start(
                    slab[:].rearrange("r (p b) -> r p b", p=NPAIRS, b=TILE),
                    x_view[:, :, b0:b0 + TILE])
                return slab

            units = [(it, t) for it in range(ntiles) for t in range(21)]
            NU = len(units)
            # slab loads run TWO batch-tiles ahead on the sync HWDGE ring.
            xts = {0: issue_load(0)}
            nc.scalar.dma_start(w1s[:], w1_dram)
            nc.scalar.dma_start(b1s[:], b1_dram)
            nc.scalar.dma_start(b2s[:], b2_dram)
            nc.scalar.dma_start(b3s[:], b3_dram)
            nc.scalar.dma_start(w2s[:], w2_dram)
            nc.scalar.dma_start(w3s[:], w3_dram)
            if ntiles > 1:
                xts[1] = issue_load(1)
            h1t = {}
            h2t = {}

            def stage_l1(k):
                it, t = units[k]
                if t == 0 and it + 2 < ntiles:
                    xts[it + 2] = issue_load(it + 2)
                slab = xts[it]
                pl = L1_PLAN[t]
                psum1 = ps12.tile([128, TILE], f32, tag="ps12", name="ps12")
                chunks = []
                for i, pr in enumerate(pl["pairs"]):
                    chunks.append((W1_COLS[(t, "pair", i)], pr["tile"]))
                for i, e in enumerate(pl["singles"]):
                    chunks.append((W1_COLS[(t, "single", i)], e["tile"]))
                nch = len(chunks)
                for ci, (c, tl) in enumerate(chunks):
                    for h in range(2):
                        nc.tensor.matmul(
                            psum1[:, 512 * h:512 * (h + 1)],
                            w1s[:, c:c + 128],
                            slab[:, TILE * tl + 512 * h:TILE * tl + 512 * (h + 1)],
                            start=(ci == 0), stop=(ci == nch - 1))
                h1 = h1p.tile([128, TILE], bf16, tag="h1", name="h1")
                evac(h1[:], psum1[:], b1s[:, t:t + 1], True, TILE)
                h1t[k] = h1

            def stage_l2(k):
                it, t = units[k]
                h1 = h1t.pop(k)
                psum2 = ps12.tile([128, TILE], f32, tag="ps12", name="ps12")
                for h in range(2):
                    nc.tensor.matmul(
                        psum2[:, 512 * h:512 * (h + 1)],
                        w2s[:, 128 * t:128 * (t + 1)],
                        h1[:, 512 * h:512 * (h + 1)],
                        start=True, stop=True)
                h2 = h2p.tile([128, TILE], bf16, tag=f"h2_{t % 4}",
                              name=f"h2_{t % 4}")
                evac(h2[:], psum2[:], b2s[:, t:t + 1], True, TILE)
                h2t[k] = h2

            def stage_l3(k):
                it, t = units[k]
                if not (t % 2 == 1 or t == 20):
                    return
                b0 = it * TILE
                tp = t // 2
                if t % 2 == 1:
                    tlo, thi = t - 1, t
                    h2lo, h2hi = h2t.pop(k - 1), h2t.pop(k)
                    rows = 128
                else:
                    tlo, thi = t, None
                    h2lo, h2hi = h2t.pop(k), None
                    rows = 64
                ot = outp.tile([128, TILE], bf16, tag="ot", name="ot")
                psum3 = ps3.tile([128, TILE], f32, tag="psum3", name="psum3")
                for h in range(2):
                    nc.tensor.matmul(
                        psum3[0:64, 512 * h:512 * (h + 1)],
                        w3s[:, 64 * tlo:64 * tlo + 64],
                        h2lo[:, 512 * h:512 * (h + 1)],
                        start=True, stop=True, skip_group_check=True)
                if thi is not None:
                    for h in range(2):
                        nc.tensor.matmul(
                            psum3[64:128, 512 * h:512 * (h + 1)],
                            w3s[:, 64 * thi:64 * thi + 64],
                            h2hi[:, 512 * h:512 * (h + 1)],
                            start=True, stop=True, skip_group_check=True)
                evac(ot[0:rows, :], psum3[0:rows, :], b3s[0:rows, tp:tp + 1],
                     False, TILE)
                nc.gpsimd.dma_start(
                    out_dram[128 * tp:128 * tp + rows, b0:b0 + TILE],
                    ot[0:rows, :])

            for k in range(NU + 4):
                if k < NU:
                    stage_l1(k)
                if 0 <= k - 2 < NU:
                    stage_l2(k - 2)
                if 0 <= k - 4 < NU:
                    stage_l3(k - 4)

    nc.compile()
    return nc


PACKED = None
_NC = None
LAST_RESULT = None


def prepare(inputs):
    """Build (once) the bass module and the per-core input maps."""
    global PACKED, _NC
    import sys
    if "/opt/trn_rl_repo" not in sys.path:
        sys.path.insert(0, "/opt/trn_rl_repo")
    x = np.asarray(inputs["x"], np.float32)
    PACKED = pack_weights(inputs)
    if _NC is None:
        _NC = build_bass_kernel()
    in_maps = []
    for core in range(NCORES):
        m = dict(PACKED)
        m["xpk"] = pack_x(x[core * BC:(core + 1) * BC])
        in_maps.append(m)
    return _NC, in_maps


def kernel(**inputs):
    global LAST_RESULT
    nc, in_maps = prepare(inputs)
    from concourse.bass_utils import run_bass_kernel_spmd
    res = run_bass_kernel_spmd(nc, in_maps, core_ids=list(range(NCORES)))
    LAST_RESULT = res
    # outf is [1344, BC] bf16 feature-major per core; unshard + transpose host-side.
    out = np.empty((B, J, D), np.float32)
    for core, r in enumerate(res.results):
        fm = np.asarray(r["outf"]).reshape(J, D, BC).astype(np.float32)
        out[core * BC:(core + 1) * BC] = fm.transpose(2, 0, 1)
    return out


# revision 4
# speedup vs baseline: 1.0843x; 1.0843x over previous
"""Trainium2 Bass kernel for the 21-joint hand-graph message-passing MLP.

Math (per sample b, per target joint t with neighbor list S_t of length n):
    g   = concat(x[b, S_t[0]], ..., x[b, S_t[n-1]])          # [n*64]
    h1  = relu(g @ W1_t + b1_t)                              # [128]
    h2  = relu(h1 @ W2_t + b2_t)                             # [128]
    out[b, t] = h2 @ W3_t + b3_t                             # [64]

Strategy (pure data parallel over 8 NeuronCores, B=65536 -> 8192/core):
  - x is pre-transposed ON THE HOST into a feature-major pair-tile layout
    xpk [15*128, BC] bf16: block p holds the 64 features of node u_p on
    partitions 0-63 and node v_p on 64-127.  The kernel reads it with ONE
    plain 3.84MB DMA per 1024-batch tile (sync HWDGE ring) -- no xbar
    transposes at all.  Host-side pair packing makes ARBITRARY node pairs
    available, so L1 runs the optimal per-target pairing: 28 genuine
    K=128 pair chunks + 13 zero-padded singles = 41 chunks (vs 47 for
    the adjacent-pair tile set), all full 128-row matmuls (partial-row
    matmuls that mix row groups in a PSUM bank can fault the device).
  - L2 is weight-stationary (w2 [128,128]); relu+bias fused into the PSUM
    evacuations, greedily load-balanced between ScalarE and VectorE.
  - L3 is w3-stationary with h2 moving at N=512: two adjacent targets
    share one 2-bank PSUM tile in disjoint col groups (concurrent
    matmuls); b3 is a per-partition bias folded into the single [128,1024]
    evacuation, which writes bf16.  Stores go out on the gpsimd SWDGE
    queues (separate from the load ring).  The feature-major [1344, BC]
    bf16 result is transposed back to [B, 21, 64] f32 on the host.
  - The PE stream is software-pipelined (L1(k) | L2(k-2) | L3(k-4)),
    continuous across batch-tile boundaries; the 2-step L2 lag hides the
    h1 PSUM-evacuation latency behind a full step of L1 matmuls.
"""

import numpy as np
import ml_dtypes

B, J, D, H1, H2 = 65536, 21, 64, 128, 128
NCORES = 8
BC = B // NCORES          # 8192 samples per core
TILE = 1024               # batch tile (psum1/psum2 = 2 PSUM banks in fp32)
NTILES = BC // TILE       # 8
TPAIRS = 11               # L3 target pairs: (0,1),(2,3),...,(18,19),(20,)

# Host-packed x pair tiles: ANY (u, v) node pair can be a tile because the
# host lays them out adjacently.  Chosen so every target's neighbor list
# splits into ceil(n/2) chunks (28 pairs + 13 singles = 41 chunks).
XTILES = [(0, 13), (1, 5), (9, 17), (0, 5), (1, 2), (3, 4), (5, 6),
          (0, 1), (7, 8), (9, 10), (11, 12), (13, 14), (15, 16),
          (17, 18), (19, 20)]
NPAIRS = len(XTILES)
PAIR_TILE = {p: i for i, p in enumerate(XTILES)}
TILE_OF = {}              # node -> (tile_idx, half), first occurrence
for _i, (_u, _v) in enumerate(XTILES):
    TILE_OF.setdefault(_u, (_i, 0))
    TILE_OF.setdefault(_v, (_i, 1))

FINGER_BASE = [4 * f + 1 for f in range(5)]
NEIGH = {
    6: [[0, 1, 5, 9, 13, 17]],
    5: [[0, 5, 6, 1, 9], [0, 9, 10, 5, 13], [0, 13, 14, 9, 17]],
    4: [[0, 1, 2, 5], [0, 17, 18, 13]],
    3: [r for b in FINGER_BASE for r in ([b, b + 1, b + 2], [b + 1, b + 2, b + 3])],
    2: [[b + 2, b + 3] for b in FINGER_BASE],
}
OUT = {
    6: [0],
    5: [5, 9, 13],
    4: [1, 17],
    3: [j for b in FINGER_BASE for j in (b + 1, b + 2)],
    2: [b + 3 for b in FINGER_BASE],
}
GROUPS = [6, 5, 4, 3, 2]

# target t -> (group n, row index within its group, neighbor list)
TARGET = {}
for n in GROUPS:
    for row, t in enumerate(OUT[n]):
        TARGET[t] = (n, row, list(NEIGH[n][row]))

# Hand-chosen optimal pairing per target (node pairs; all are in XTILES).
PAIRING = {
    0: [(0, 13), (1, 5), (9, 17)],
    1: [(0, 5), (1, 2)],
    2: [(1, 2)],
    3: [(3, 4)],
    4: [(3, 4)],
    5: [(5, 6), (0, 1)],
    6: [(5, 6)],
    7: [(7, 8)],
    8: [(7, 8)],
    9: [(9, 10), (0, 13)],
    10: [(9, 10)],
    11: [(11, 12)],
    12: [(11, 12)],
    13: [(13, 14), (9, 17)],
    14: [(13, 14)],
    15: [(15, 16)],
    16: [(15, 16)],
    17: [(17, 18), (0, 13)],
    18: [(17, 18)],
    19: [(19, 20)],
    20: [(19, 20)],
}


def build_l1_plan():
    plan = {}
    for t in range(21):
        n, _, S = TARGET[t]
        used = [False] * n
        pairs = []
        for (u, v) in PAIRING[t]:
            i, k = S.index(u), S.index(v)
            assert not used[i] and not used[k]
            used[i] = used[k] = True
            pairs.append(dict(tile=PAIR_TILE[(u, v)], pos0=i, pos1=k))
        singles = []
        for i in range(n):
            if not used[i]:
                tile_idx, half = TILE_OF[S[i]]
                singles.append(dict(tile=tile_idx, pos=i, half=half))
        assert len(singles) <= 1
        plan[t] = dict(pairs=pairs, singles=singles)
    return plan


L1_PLAN = build_l1_plan()


def assign_w1_cols():
    cols = {}
    col = 0
    for t in range(21):
        p = L1_PLAN[t]
        for i, _ in enumerate(p["pairs"]):
            cols[(t, "pair", i)] = col
            col += 128
        for i, _ in enumerate(p["singles"]):
            cols[(t, "single", i)] = col
            col += 128
    return cols, col


W1_COLS, W1_NCOL = assign_w1_cols()      # 41 * 128 = 5248


def pack_weights(inputs):
    """Host-side prep: permute/pack all weights into flat bf16/f32 arrays."""
    bf16 = ml_dtypes.bfloat16
    w1p = np.zeros((128, W1_NCOL), np.float32)
    for t in range(21):
        n, row, S = TARGET[t]
        W1 = np.asarray(inputs[f"w1_g{n}"][row], np.float32)  # [n*64, 128]
        p = L1_PLAN[t]
        for i, pr in enumerate(p["pairs"]):
            c = W1_COLS[(t, "pair", i)]
            w1p[0:64, c:c + 128] = W1[64 * pr["pos0"]:64 * pr["pos0"] + 64]
            w1p[64:128, c:c + 128] = W1[64 * pr["pos1"]:64 * pr["pos1"] + 64]
        for i, e in enumerate(p["singles"]):
            c = W1_COLS[(t, "single", i)]
            half = e["half"]
            w1p[64 * half:64 * half + 64, c:c + 128] = \
                W1[64 * e["pos"]:64 * e["pos"] + 64]
    w2p = np.zeros((128, 128 * 21), np.float32)
    w3p = np.zeros((128, 64 * 21), np.float32)
    b1p = np.zeros((128, 21), np.float32)
    b2p = np.zeros((128, 21), np.float32)
    b3p = np.zeros((128, TPAIRS), np.float32)   # per-partition bias, paired
    for t in range(21):
        n, row, _ = TARGET[t]
        w2p[:, 128 * t:128 * (t + 1)] = np.asarray(inputs[f"w2_g{n}"][row])
        w3p[:, 64 * t:64 * (t + 1)] = np.asarray(inputs[f"w3_g{n}"][row])
        b1p[:, t] = np.asarray(inputs[f"b1_g{n}"][row])
        b2p[:, t] = np.asarray(inputs[f"b2_g{n}"][row])
        b3p[64 * (t % 2):64 * (t % 2) + 64, t // 2] = \
            np.asarray(inputs[f"b3_g{n}"][row])
    return dict(
        w1p=w1p.astype(bf16), w2p=w2p.astype(bf16), w3p=w3p.astype(bf16),
        b1p=b1p, b2p=b2p, b3p=b3p,
    )


def pack_x(x):
    """x [Bn, 21, 64] f32 -> xpk [NPAIRS*128, Bn] bf16 (feature-major pairs)."""
    bf16 = ml_dtypes.bfloat16
    Bn = x.shape[0]
    xt = np.ascontiguousarray(
        np.asarray(x, np.float32).astype(bf16).transpose(1, 2, 0))  # [21,64,Bn]
    xpk = np.empty((NPAIRS * 128, Bn), bf16)
    for p, (u, v) in enumerate(XTILES):
        xpk[128 * p:128 * p + 64] = xt[u]
        xpk[128 * p + 64:128 * p + 128] = xt[v]
    return xpk


def numpy_emulate(inputs, x):
    """Bit-layout-faithful numpy model of the HW kernel (minus PSUM rounding):
    validates the chunk plan / packing / L3 pairing offline."""
    bf16 = ml_dtypes.bfloat16
    packed = pack_weights(inputs)
    xpk = pack_x(x)
    Bn = x.shape[0]
    out = np.zeros((Bn, 21, 64), np.float32)
    for t in range(21):
        psum1 = np.zeros((128, Bn), np.float32)
        pl = L1_PLAN[t]
        chunks = [("pair", i, pr["tile"]) for i, pr in enumerate(pl["pairs"])]
        chunks += [("single", i, e["tile"]) for i, e in enumerate(pl["singles"])]
        for kind, i, tl in chunks:
            c = W1_COLS[(t, kind, i)]
            lhsT = packed["w1p"][:, c:c + 128].astype(np.float32)
            rhs = xpk[128 * tl:128 * tl + 128].astype(np.float32)
            psum1 += lhsT.T @ rhs
        h1 = np.maximum(psum1 + packed["b1p"][:, t:t + 1], 0).astype(bf16)
        w2 = packed["w2p"][:, 128 * t:128 * (t + 1)].astype(np.float32)
        psum2 = w2.T @ h1.astype(np.float32)
        h2 = np.maximum(psum2 + packed["b2p"][:, t:t + 1], 0).astype(bf16)
        w3 = packed["w3p"][:, 64 * t:64 * (t + 1)].astype(np.float32)
        b3 = packed["b3p"][64 * (t % 2):64 * (t % 2) + 64, t // 2]
        o = (w3.T @ h2.astype(np.float32) + b3[:, None]).astype(bf16)
        out[:, t] = o.T.astype(np.float32)
    return out


# ---------------------------------------------------------------------------
# Bass kernel
# ---------------------------------------------------------------------------

def build_bass_kernel(num_devices=NCORES, bc=BC):
    import concourse.bass as bass
    import concourse.tile as tile
    from concourse import bacc, mybir

    bf16 = mybir.dt.bfloat16
    f32 = mybir.dt.float32
    Relu = mybir.ActivationFunctionType.Relu
    Ident = mybir.ActivationFunctionType.Identity
    Alu = mybir.AluOpType
    ntiles = bc // TILE

    nc = bacc.Bacc("TRN2", target_bir_lowering=False, debug=False,
                   num_devices=num_devices)
    x_dram = nc.dram_tensor("xpk", [NPAIRS * 128, bc], bf16,
                            kind="ExternalInput").ap()
    out_dram = nc.dram_tensor("outf", [J * D, bc], bf16,
                              kind="ExternalOutput").ap()
    w1_dram = nc.dram_tensor("w1p", [128, W1_NCOL], bf16, kind="ExternalInput").ap()
    w2_dram = nc.dram_tensor("w2p", [128, 128 * 21], bf16, kind="ExternalInput").ap()
    w3_dram = nc.dram_tensor("w3p", [128, 64 * 21], bf16, kind="ExternalInput").ap()
    b1_dram = nc.dram_tensor("b1p", [128, 21], f32, kind="ExternalInput").ap()
    b2_dram = nc.dram_tensor("b2p", [128, 21], f32, kind="ExternalInput").ap()
    b3_dram = nc.dram_tensor("b3p", [128, TPAIRS], f32, kind="ExternalInput").ap()

    with tile.TileContext(nc) as tc:
        with (
            tc.tile_pool(name="wpool", bufs=1) as wpool,
            tc.tile_pool(name="xtp", bufs=3) as xtp,
            tc.tile_pool(name="h1p", bufs=3) as h1p,
            tc.tile_pool(name="h2p", bufs=1) as h2p,
            tc.tile_pool(name="outp", bufs=6) as outp,
            tc.tile_pool(name="psp", bufs=4, space="PSUM") as psp,
        ):
            w1s = wpool.tile([128, W1_NCOL], bf16, name="w1s")
            w2s = wpool.tile([128, 128 * 21], bf16, name="w2s")
            w3s = wpool.tile([128, 64 * 21], bf16, name="w3s")
            b1s = wpool.tile([128, 21], f32, name="b1s")
            b2s = wpool.tile([128, 21], f32, name="b2s")
            b3s = wpool.tile([128, TPAIRS], f32, name="b3s")
            dummy = wpool.tile([128, 640], bf16, name="dummy")

            # greedy ScalarE/VectorE balance for the PSUM evacuations,
            # using HW-measured op costs (ACT 1335ns / DVE 1536ns @ FD=1024)
            ev_time = [0.0, 0.0]          # ns on [ACT, DVE]

            def evac(dst, src, bias, relu, fd):
                act_cost = (352.0 + fd) * 1.16 / 1.2
                dve_cost = (450.0 + fd * 1.06) / 0.96
                if ev_time[0] + act_cost <= ev_time[1] + dve_cost:
                    ev_time[0] += act_cost
                    nc.scalar.activation(dst, src, Relu if relu else Ident,
                                         bias=bias, scale=1.0)
                else:
                    ev_time[1] += dve_cost
                    if relu:
                        nc.vector.tensor_scalar(dst, src, bias, 0.0,
                                                Alu.add, Alu.max)
                    else:
                        nc.vector.tensor_scalar(dst, src, bias, None, Alu.add)

            def mm2(out_ap, wcol_lo, wcol_hi, wtile, rhs_lo, rhs_hi,
                    start, stop, base=0):
                """One N=512 slot as two concurrent 64-col tile matmuls.
                All matmuls in the kernel use (128,64) tiling so the PE
                never switches tiling mode (mode switches drain the array
                and serialize; uniform mode lets T0/T1 run concurrently)."""
                nc.tensor.matmul(out_ap[base:base + 64, :], wtile[:, wcol_lo],
                                 rhs_lo, start=start, stop=stop,
                                 skip_group_check=True)
                nc.tensor.matmul(out_ap[base + 64:base + 128, :],
                                 wtile[:, wcol_hi],
                                 rhs_hi, start=start, stop=stop,
                                 skip_group_check=True)

            def issue_load(it):
                b0 = it * TILE
                slab = xtp.tile([128, NPAIRS * TILE], bf16, tag="slab",
                                name="slab")
                for p in range(NPAIRS):
                    nc.sync.dma_start(
                        slab[:, TILE * p:TILE * (p + 1)],
                        x_dram[128 * p:128 * (p + 1), b0:b0 + TILE])
                return slab

            units = [(it, t) for it in range(ntiles) for t in range(21)]
            NU = len(units)
            # per-pair slab loads run TWO batch-tiles ahead on the sync
            # HWDGE ring, in first-use order; w1 is split so the columns
            # for the first targets arrive first.
            xts = {0: issue_load(0)}
            W1SPLIT = 2048
            nc.scalar.dma_start(w1s[:, 0:W1SPLIT], w1_dram[:, 0:W1SPLIT])
            nc.scalar.dma_start(b1s[:], b1_dram)
            nc.scalar.dma_start(w2s[:], w2_dram)
            nc.scalar.dma_start(b2s[:], b2_dram)
            nc.scalar.dma_start(w3s[:], w3_dram)
            nc.scalar.dma_start(b3s[:], b3_dram)
            nc.scalar.dma_start(w1s[:, W1SPLIT:], w1_dram[:, W1SPLIT:])
            if ntiles > 1:
                xts[1] = issue_load(1)
            h1t = {}
            h2t = {}

            # PE warm-up: dummy (128,64)-tiled matmuls with no data deps
            # keep the PE busy through the initial load phase so the HAM
            # clock-gate is released before the first real matmul.
            nc.vector.memset(dummy[:], 0.0)
            warm = psp.tile([128, TILE], f32, tag="ps", name="ps")
            for _ in range(8):
                mm2(warm[:, 0:512], slice(0, 64), slice(64, 128), dummy,
                    dummy[:, 128:640], dummy[:, 128:640], True, True)

            def stage_l1(k):
                it, t = units[k]
                if t == 0 and it + 2 < ntiles:
                    xts[it + 2] = issue_load(it + 2)
                slab = xts[it]
                pl = L1_PLAN[t]
                psum1 = psp.tile([128, TILE], f32, tag="ps", name="ps")
                chunks = []
                for i, pr in enumerate(pl["pairs"]):
                    chunks.append((W1_COLS[(t, "pair", i)], pr["tile"]))
                for i, e in enumerate(pl["singles"]):
                    chunks.append((W1_COLS[(t, "single", i)], e["tile"]))
                nch = len(chunks)
                for ci, (c, tl) in enumerate(chunks):
                    for h in range(2):
                        rhs = slab[:, TILE * tl + 512 * h:TILE * tl + 512 * (h + 1)]
                        mm2(psum1[:, 512 * h:512 * (h + 1)],
                            slice(c, c + 64), slice(c + 64, c + 128), w1s,
                            rhs, rhs, ci == 0, ci == nch - 1)
                h1 = h1p.tile([128, TILE], bf16, tag="h1", name="h1")
                evac(h1[:], psum1[:], b1s[:, t:t + 1], True, TILE)
                h1t[k] = h1

            def stage_l2(k):
                it, t = units[k]
                h1 = h1t.pop(k)
                psum2 = psp.tile([128, TILE], f32, tag="ps", name="ps")
                for h in range(2):
                    rhs = h1[:, 512 * h:512 * (h + 1)]
                    mm2(psum2[:, 512 * h:512 * (h + 1)],
                        slice(128 * t, 128 * t + 64),
                        slice(128 * t + 64, 128 * (t + 1)), w2s,
                        rhs, rhs, True, True)
                h2 = h2p.tile([128, TILE], bf16, tag=f"h2_{t % 4}",
                              name=f"h2_{t % 4}")
                evac(h2[:], psum2[:], b2s[:, t:t + 1], True, TILE)
                h2t[k] = h2

            def stage_l3(k):
                it, t = units[k]
                if not (t % 2 == 1 or t == 20):
                    return
                b0 = it * TILE
                tp = t // 2
                if t % 2 == 1:
                    tlo, thi = t - 1, t
                    h2lo, h2hi = h2t.pop(k - 1), h2t.pop(k)
                    rows = 128
                else:
                    tlo, thi = t, None
                    h2lo, h2hi = h2t.pop(k), None
                    rows = 64
                ot = outp.tile([128, TILE], bf16, tag="ot", name="ot")
                psum3 = psp.tile([128, TILE], f32, tag="ps", name="ps")
                for h in range(2):
                    if thi is not None:
                        # tlo on col tile T0, thi on T1 — concurrent
                        mm2(psum3[:, 512 * h:512 * (h + 1)],
                            slice(64 * tlo, 64 * tlo + 64),
                            slice(64 * thi, 64 * thi + 64), w3s,
                            h2lo[:, 512 * h:512 * (h + 1)],
                            h2hi[:, 512 * h:512 * (h + 1)], True, True)
                    else:
                        nc.tensor.matmul(
                            psum3[0:64, 512 * h:512 * (h + 1)],
                            w3s[:, 64 * tlo:64 * tlo + 64],
                            h2lo[:, 512 * h:512 * (h + 1)],
                            start=True, stop=True, skip_group_check=True)
                evac(ot[0:rows, :], psum3[0:rows, :], b3s[0:rows, tp:tp + 1],
                     False, TILE)
                nc.scalar.dma_start(
                    out_dram[128 * tp:128 * tp + rows, b0:b0 + TILE],
                    ot[0:rows, :])

            for k in range(NU + 4):
                if k < NU:
                    stage_l1(k)
                if 0 <= k - 2 < NU:
                    stage_l2(k - 2)
                if 0 <= k - 4 < NU:
                    stage_l3(k - 4)

    nc.compile()
    return nc


PACKED = None
_NC = None
LAST_RESULT = None


def prepare(inputs):
    """Build (once) the bass module and the per-core input maps."""
    global PACKED, _NC
    import sys
    if "/opt/trn_rl_repo" not in sys.path:
        sys.path.insert(0, "/opt/trn_rl_repo")
    x = np.asarray(inputs["x"], np.float32)
    PACKED = pack_weights(inputs)
    if _NC is None:
        _NC = build_bass_kernel()
    in_maps = []
    for core in range(NCORES):
        m = dict(PACKED)
        m["xpk"] = pack_x(x[core * BC:(core + 1) * BC])
        in_maps.append(m)
    return _NC, in_maps


def kernel(**inputs):
    global LAST_RESULT
    nc, in_maps = prepare(inputs)
    from concourse.bass_utils import run_bass_kernel_spmd
    res = run_bass_kernel_spmd(nc, in_maps, core_ids=list(range(NCORES)))
    LAST_RESULT = res
    # outf is [1344, BC] bf16 feature-major per core; unshard + transpose host-side.
    out = np.empty((B, J, D), np.float32)
    for core, r in enumerate(res.results):
        fm = np.asarray(r["outf"]).reshape(J, D, BC).astype(np.float32)
        out[core * BC:(core + 1) * BC] = fm.transpose(2, 0, 1)
    return out


# revision 9
# speedup vs baseline: 1.3108x; 1.2089x over previous
"""Trainium2 Bass kernel for the 21-joint hand-graph message-passing MLP.

Math (per sample b, per target joint t with neighbor list S_t of length n):
    g   = concat(x[b, S_t[0]], ..., x[b, S_t[n-1]])          # [n*64]
    h1  = relu(g @ W1_t + b1_t)                              # [128]
    h2  = relu(h1 @ W2_t + b2_t)                             # [128]
    out[b, t] = h2 @ W3_t + b3_t                             # [64]

Strategy (pure data parallel over 8 NeuronCores, B=65536 -> 8192/core):
  - x is pre-transposed ON THE HOST into a feature-major pair-tile layout
    xpk [15*128, BC] bf16: block p holds the 64 features of node u_p on
    partitions 0-63 and node v_p on 64-127.  The kernel reads it with ONE
    plain 3.84MB DMA per 1024-batch tile (sync HWDGE ring) -- no xbar
    transposes at all.  Host-side pair packing makes ARBITRARY node pairs
    available, so L1 runs the optimal per-target pairing: 28 genuine
    K=128 pair chunks + 13 zero-padded singles = 41 chunks (vs 47 for
    the adjacent-pair tile set), all full 128-row matmuls (partial-row
    matmuls that mix row groups in a PSUM bank can fault the device).
  - L2 is weight-stationary (w2 [128,128]); relu+bias fused into the PSUM
    evacuations, greedily load-balanced between ScalarE and VectorE.
  - L3 is w3-stationary with h2 moving at N=512: two adjacent targets
    share one 2-bank PSUM tile in disjoint col groups (concurrent
    matmuls); b3 is a per-partition bias folded into the single [128,1024]
    evacuation, which writes bf16.  Stores go out on the gpsimd SWDGE
    queues (separate from the load ring).  The feature-major [1344, BC]
    bf16 result is transposed back to [B, 21, 64] f32 on the host.
  - The PE stream is software-pipelined (L1(k) | L2(k-2) | L3(k-4)),
    continuous across batch-tile boundaries; the 2-step L2 lag hides the
    h1 PSUM-evacuation latency behind a full step of L1 matmuls.
"""

import numpy as np
import ml_dtypes

B, J, D, H1, H2 = 65536, 21, 64, 128, 128
NCORES = 8
BC = B // NCORES          # 8192 samples per core
TILE = 1024               # batch tile (psum1/psum2 = 2 PSUM banks in fp32)
NTILES = BC // TILE       # 8
TPAIRS = 11               # L3 target pairs: (0,1),(2,3),...,(18,19),(20,)

# Host-packed x pair tiles: ANY (u, v) node pair can be a tile because the
# host lays them out adjacently.  Chosen so every target's neighbor list
# splits into ceil(n/2) chunks (28 pairs + 13 singles = 41 chunks).
XTILES = [(0, 13), (1, 5), (9, 17), (0, 5), (1, 2), (3, 4), (5, 6),
          (0, 1), (7, 8), (9, 10), (11, 12), (13, 14), (15, 16),
          (17, 18), (19, 20)]
NPAIRS = len(XTILES)
PAIR_TILE = {p: i for i, p in enumerate(XTILES)}
TILE_OF = {}              # node -> (tile_idx, half), first occurrence
for _i, (_u, _v) in enumerate(XTILES):
    TILE_OF.setdefault(_u, (_i, 0))
    TILE_OF.setdefault(_v, (_i, 1))

FINGER_BASE = [4 * f + 1 for f in range(5)]
NEIGH = {
    6: [[0, 1, 5, 9, 13, 17]],
    5: [[0, 5, 6, 1, 9], [0, 9, 10, 5, 13], [0, 13, 14, 9, 17]],
    4: [[0, 1, 2, 5], [0, 17, 18, 13]],
    3: [r for b in FINGER_BASE for r in ([b, b + 1, b + 2], [b + 1, b + 2, b + 3])],
    2: [[b + 2, b + 3] for b in FINGER_BASE],
}
OUT = {
    6: [0],
    5: [5, 9, 13],
    4: [1, 17],
    3: [j for b in FINGER_BASE for j in (b + 1, b + 2)],
    2: [b + 3 for b in FINGER_BASE],
}
GROUPS = [6, 5, 4, 3, 2]

# target t -> (group n, row index within its group, neighbor list)
TARGET = {}
for n in GROUPS:
    for row, t in enumerate(OUT[n]):
        TARGET[t] = (n, row, list(NEIGH[n][row]))

# Hand-chosen optimal pairing per target (node pairs; all are in XTILES).
PAIRING = {
    0: [(0, 13), (1, 5), (9, 17)],
    1: [(0, 5), (1, 2)],
    2: [(1, 2)],
    3: [(3, 4)],
    4: [(3, 4)],
    5: [(5, 6), (0, 1)],
    6: [(5, 6)],
    7: [(7, 8)],
    8: [(7, 8)],
    9: [(9, 10), (0, 13)],
    10: [(9, 10)],
    11: [(11, 12)],
    12: [(11, 12)],
    13: [(13, 14), (9, 17)],
    14: [(13, 14)],
    15: [(15, 16)],
    16: [(15, 16)],
    17: [(17, 18), (0, 13)],
    18: [(17, 18)],
    19: [(19, 20)],
    20: [(19, 20)],
}


def build_l1_plan():
    plan = {}
    for t in range(21):
        n, _, S = TARGET[t]
        used = [False] * n
        pairs = []
        for (u, v) in PAIRING[t]:
            i, k = S.index(u), S.index(v)
            assert not used[i] and not used[k]
            used[i] = used[k] = True
            pairs.append(dict(tile=PAIR_TILE[(u, v)], pos0=i, pos1=k))
        singles = []
        for i in range(n):
            if not used[i]:
                tile_idx, half = TILE_OF[S[i]]
                singles.append(dict(tile=tile_idx, pos=i, half=half))
        assert len(singles) <= 1
        plan[t] = dict(pairs=pairs, singles=singles)
    return plan


L1_PLAN = build_l1_plan()


def assign_w1_cols():
    cols = {}
    col = 0
    for t in range(21):
        p = L1_PLAN[t]
        for i, _ in enumerate(p["pairs"]):
            cols[(t, "pair", i)] = col
            col += 128
        for i, _ in enumerate(p["singles"]):
            cols[(t, "single", i)] = col
            col += 128
    return cols, col


W1_COLS, W1_NCOL = assign_w1_cols()      # 41 * 128 = 5248


def pack_weights(inputs):
    """Host-side prep: permute/pack all weights into flat bf16/f32 arrays."""
    bf16 = ml_dtypes.bfloat16
    w1p = np.zeros((128, W1_NCOL), np.float32)
    for t in range(21):
        n, row, S = TARGET[t]
        W1 = np.asarray(inputs[f"w1_g{n}"][row], np.float32)  # [n*64, 128]
        p = L1_PLAN[t]
        for i, pr in enumerate(p["pairs"]):
            c = W1_COLS[(t, "pair", i)]
            w1p[0:64, c:c + 128] = W1[64 * pr["pos0"]:64 * pr["pos0"] + 64]
            w1p[64:128, c:c + 128] = W1[64 * pr["pos1"]:64 * pr["pos1"] + 64]
        for i, e in enumerate(p["singles"]):
            c = W1_COLS[(t, "single", i)]
            half = e["half"]
            w1p[64 * half:64 * half + 64, c:c + 128] = \
                W1[64 * e["pos"]:64 * e["pos"] + 64]
    w2p = np.zeros((128, 128 * 21), np.float32)
    w3p = np.zeros((128, 64 * 21), np.float32)
    b1p = np.zeros((128, 21), np.float32)
    b2p = np.zeros((128, 21), np.float32)
    b3p = np.zeros((128, TPAIRS), np.float32)   # per-partition bias, paired
    for t in range(21):
        n, row, _ = TARGET[t]
        w2p[:, 128 * t:128 * (t + 1)] = np.asarray(inputs[f"w2_g{n}"][row])
        w3p[:, 64 * t:64 * (t + 1)] = np.asarray(inputs[f"w3_g{n}"][row])
        b1p[:, t] = np.asarray(inputs[f"b1_g{n}"][row])
        b2p[:, t] = np.asarray(inputs[f"b2_g{n}"][row])
        b3p[64 * (t % 2):64 * (t % 2) + 64, t // 2] = \
            np.asarray(inputs[f"b3_g{n}"][row])
    return dict(
        w1p=w1p.astype(bf16), w2p=w2p.astype(bf16), w3p=w3p.astype(bf16),
        b1p=b1p, b2p=b2p, b3p=b3p,
    )


def pack_x(x):
    """x [Bn, 21, 64] f32 -> xpk [NPAIRS*128, Bn] bf16 (feature-major pairs)."""
    bf16 = ml_dtypes.bfloat16
    Bn = x.shape[0]
    xt = np.ascontiguousarray(
        np.asarray(x, np.float32).astype(bf16).transpose(1, 2, 0))  # [21,64,Bn]
    xpk = np.empty((NPAIRS * 128, Bn), bf16)
    for p, (u, v) in enumerate(XTILES):
        xpk[128 * p:128 * p + 64] = xt[u]
        xpk[128 * p + 64:128 * p + 128] = xt[v]
    return xpk


def numpy_emulate(inputs, x):
    """Bit-layout-faithful numpy model of the HW kernel (minus PSUM rounding):
    validates the chunk plan / packing / L3 pairing offline."""
    bf16 = ml_dtypes.bfloat16
    packed = pack_weights(inputs)
    xpk = pack_x(x)
    Bn = x.shape[0]
    out = np.zeros((Bn, 21, 64), np.float32)
    for t in range(21):
        psum1 = np.zeros((128, Bn), np.float32)
        pl = L1_PLAN[t]
        chunks = [("pair", i, pr["tile"]) for i, pr in enumerate(pl["pairs"])]
        chunks += [("single", i, e["tile"]) for i, e in enumerate(pl["singles"])]
        for kind, i, tl in chunks:
            c = W1_COLS[(t, kind, i)]
            lhsT = packed["w1p"][:, c:c + 128].astype(np.float32)
            rhs = xpk[128 * tl:128 * tl + 128].astype(np.float32)
            psum1 += lhsT.T @ rhs
        h1 = np.maximum(psum1 + packed["b1p"][:, t:t + 1], 0).astype(bf16)
        w2 = packed["w2p"][:, 128 * t:128 * (t + 1)].astype(np.float32)
        psum2 = w2.T @ h1.astype(np.float32)
        h2 = np.maximum(psum2 + packed["b2p"][:, t:t + 1], 0).astype(bf16)
        w3 = packed["w3p"][:, 64 * t:64 * (t + 1)].astype(np.float32)
        b3 = packed["b3p"][64 * (t % 2):64 * (t % 2) + 64, t // 2]
        o = (w3.T @ h2.astype(np.float32) + b3[:, None]).astype(bf16)
        out[:, t] = o.T.astype(np.float32)
    return out


# ---------------------------------------------------------------------------
# Bass kernel
# ---------------------------------------------------------------------------

def build_bass_kernel(num_devices=NCORES, bc=BC):
    import concourse.bass as bass
    import concourse.tile as tile
    from concourse import bacc, mybir

    bf16 = mybir.dt.bfloat16
    f32 = mybir.dt.float32
    Relu = mybir.ActivationFunctionType.Relu
    Ident = mybir.ActivationFunctionType.Identity
    Alu = mybir.AluOpType
    ntiles = bc // TILE

    nc = bacc.Bacc("TRN2", target_bir_lowering=False, debug=False,
                   num_devices=num_devices)
    x_dram = nc.dram_tensor("xpk", [NPAIRS * 128, bc], bf16,
                            kind="ExternalInput").ap()
    out_dram = nc.dram_tensor("outf", [J * D, bc], bf16,
                              kind="ExternalOutput").ap()
    w1_dram = nc.dram_tensor("w1p", [128, W1_NCOL], bf16, kind="ExternalInput").ap()
    w2_dram = nc.dram_tensor("w2p", [128, 128 * 21], bf16, kind="ExternalInput").ap()
    w3_dram = nc.dram_tensor("w3p", [128, 64 * 21], bf16, kind="ExternalInput").ap()
    b1_dram = nc.dram_tensor("b1p", [128, 21], f32, kind="ExternalInput").ap()
    b2_dram = nc.dram_tensor("b2p", [128, 21], f32, kind="ExternalInput").ap()
    b3_dram = nc.dram_tensor("b3p", [128, TPAIRS], f32, kind="ExternalInput").ap()
    # [128, NPAIRS, bc] view for the one-DMA-per-batch-tile slab load
    x_view = x_dram.rearrange("(p r) b -> r p b", p=NPAIRS, r=128)

    with tile.TileContext(nc) as tc:
        with (
            tc.tile_pool(name="wpool", bufs=1) as wpool,
            tc.tile_pool(name="xtp", bufs=3) as xtp,
            tc.tile_pool(name="h1p", bufs=3) as h1p,
            tc.tile_pool(name="h2p", bufs=1) as h2p,
            tc.tile_pool(name="outp", bufs=6) as outp,
            tc.tile_pool(name="psp", bufs=4, space="PSUM") as psp,
        ):
            w1s = wpool.tile([128, W1_NCOL], bf16, name="w1s")
            w2s = wpool.tile([128, 128 * 21], bf16, name="w2s")
            w3s = wpool.tile([128, 64 * 21], bf16, name="w3s")
            b1s = wpool.tile([128, 21], f32, name="b1s")
            b2s = wpool.tile([128, 21], f32, name="b2s")
            b3s = wpool.tile([128, TPAIRS], f32, name="b3s")
            dummy = wpool.tile([128, 640], bf16, name="dummy")

            # greedy ScalarE/VectorE balance for the PSUM evacuations,
            # using HW-measured op costs (ACT 1335ns / DVE 1536ns @ FD=1024)
            ev_time = [0.0, 0.0]          # ns on [ACT, DVE]

            def evac(dst, src, bias, relu, fd):
                act_cost = (312.0 + fd) / 1.2
                dve_cost = (210.0 + fd * 1.05) / 0.96
                if ev_time[0] + act_cost <= ev_time[1] + dve_cost:
                    ev_time[0] += act_cost
                    nc.scalar.activation(dst, src, Relu if relu else Ident,
                                         bias=bias, scale=1.0)
                else:
                    ev_time[1] += dve_cost
                    if relu:
                        nc.vector.tensor_scalar(dst, src, bias, 0.0,
                                                Alu.add, Alu.max)
                    else:
                        nc.vector.tensor_scalar(dst, src, bias, None, Alu.add)

            def mm2(out_ap, wcol_lo, wcol_hi, wtile, rhs_lo, rhs_hi,
                    start, stop, base=0):
                """One N=512 slot as two concurrent 64-col tile matmuls.
                All matmuls in the kernel use (128,64) tiling so the PE
                never switches tiling mode (mode switches drain the array
                and serialize; uniform mode lets T0/T1 run concurrently)."""
                nc.tensor.matmul(out_ap[base:base + 64, :], wtile[:, wcol_lo],
                                 rhs_lo, start=start, stop=stop,
                                 skip_group_check=True)
                nc.tensor.matmul(out_ap[base + 64:base + 128, :],
                                 wtile[:, wcol_hi],
                                 rhs_hi, start=start, stop=stop,
                                 skip_group_check=True)

            def issue_load(it, split=False):
                """ONE big DMA per batch-tile: a burst of small DMAs
                backlogs the HWDGE ring and the framework's DMA-semaphore
                reuse fences in the PE queue then stall on them.  The
                single DMA is issued two tiles ahead, so any fence on it
                is long-satisfied.  Tile 0 is split so the pair tiles the
                first targets need arrive first."""
                b0 = it * TILE
                slab = xtp.tile([128, NPAIRS * TILE], bf16, tag="slab",
                                name="slab")
                slab3 = slab[:].rearrange("r (p b) -> r p b", p=NPAIRS, b=TILE)
                if split:
                    for p0, p1 in ((0, 3), (3, 6), (6, 10), (10, NPAIRS)):
                        nc.sync.dma_start(slab3[:, p0:p1, :],
                                          x_view[:, p0:p1, b0:b0 + TILE])
                else:
                    nc.sync.dma_start(slab3, x_view[:, :, b0:b0 + TILE])
                return slab

            units = [(it, t) for it in range(ntiles) for t in range(21)]
            NU = len(units)
            # per-pair slab loads run TWO batch-tiles ahead on the sync
            # HWDGE ring, in first-use order; w1 is split so the columns
            # for the first targets arrive first.
            xts = {0: issue_load(0, split=True)}
            W1SPLIT = 2048
            nc.scalar.dma_start(w1s[:, 0:W1SPLIT], w1_dram[:, 0:W1SPLIT])
            nc.scalar.dma_start(b1s[:], b1_dram)
            nc.scalar.dma_start(w2s[:], w2_dram)
            nc.scalar.dma_start(b2s[:], b2_dram)
            nc.scalar.dma_start(w3s[:], w3_dram)
            nc.scalar.dma_start(b3s[:], b3_dram)
            nc.scalar.dma_start(w1s[:, W1SPLIT:], w1_dram[:, W1SPLIT:])
            if ntiles > 1:
                xts[1] = issue_load(1)
            h1t = {}
            h2t = {}

            # PE warm-up: dummy (128,64)-tiled matmuls with no data deps
            # keep the PE busy through the initial load phase so the HAM
            # clock-gate is released before the first real matmul.
            nc.vector.memset(dummy[:], 0.0)
            warm = psp.tile([128, TILE], f32, tag="ps", name="ps")
            for _ in range(8):
                mm2(warm[:, 0:512], slice(0, 64), slice(64, 128), dummy,
                    dummy[:, 128:640], dummy[:, 128:640], True, True)

            def stage_l1(k):
                it, t = units[k]
                if t == 0 and it + 2 < ntiles:
                    xts[it + 2] = issue_load(it + 2)
                slab = xts[it]
                pl = L1_PLAN[t]
                psum1 = psp.tile([128, TILE], f32, tag="ps", name="ps")
                chunks = []
                for i, pr in enumerate(pl["pairs"]):
                    chunks.append((W1_COLS[(t, "pair", i)], pr["tile"]))
                for i, e in enumerate(pl["singles"]):
                    chunks.append((W1_COLS[(t, "single", i)], e["tile"]))
                nch = len(chunks)
                for ci, (c, tl) in enumerate(chunks):
                    for h in range(2):
                        rhs = slab[:, TILE * tl + 512 * h:TILE * tl + 512 * (h + 1)]
                        mm2(psum1[:, 512 * h:512 * (h + 1)],
                            slice(c, c + 64), slice(c + 64, c + 128), w1s,
                            rhs, rhs, ci == 0, ci == nch - 1)
                h1 = h1p.tile([128, TILE], bf16, tag="h1", name="h1")
                evac(h1[:], psum1[:], b1s[:, t:t + 1], True, TILE)
                h1t[k] = h1

            def stage_l2(k):
                it, t = units[k]
                h1 = h1t.pop(k)
                psum2 = psp.tile([128, TILE], f32, tag="ps", name="ps")
                for h in range(2):
                    rhs = h1[:, 512 * h:512 * (h + 1)]
                    mm2(psum2[:, 512 * h:512 * (h + 1)],
                        slice(128 * t, 128 * t + 64),
                        slice(128 * t + 64, 128 * (t + 1)), w2s,
                        rhs, rhs, True, True)
                h2 = h2p.tile([128, TILE], bf16, tag=f"h2_{t % 4}",
                              name=f"h2_{t % 4}")
                evac(h2[:], psum2[:], b2s[:, t:t + 1], True, TILE)
                h2t[k] = h2

            def stage_l3(k):
                it, t = units[k]
                if not (t % 2 == 1 or t == 20):
                    return
                b0 = it * TILE
                tp = t // 2
                if t % 2 == 1:
                    tlo, thi = t - 1, t
                    h2lo, h2hi = h2t.pop(k - 1), h2t.pop(k)
                    rows = 128
                else:
                    tlo, thi = t, None
                    h2lo, h2hi = h2t.pop(k), None
                    rows = 64
                ot = outp.tile([128, TILE], bf16, tag="ot", name="ot")
                psum3 = psp.tile([128, TILE], f32, tag="ps", name="ps")
                for h in range(2):
                    if thi is not None:
                        # tlo on col tile T0, thi on T1 — concurrent
                        mm2(psum3[:, 512 * h:512 * (h + 1)],
                            slice(64 * tlo, 64 * tlo + 64),
                            slice(64 * thi, 64 * thi + 64), w3s,
                            h2lo[:, 512 * h:512 * (h + 1)],
                            h2hi[:, 512 * h:512 * (h + 1)], True, True)
                    else:
                        nc.tensor.matmul(
                            psum3[0:64, 512 * h:512 * (h + 1)],
                            w3s[:, 64 * tlo:64 * tlo + 64],
                            h2lo[:, 512 * h:512 * (h + 1)],
                            start=True, stop=True, skip_group_check=True)
                evac(ot[0:rows, :], psum3[0:rows, :], b3s[0:rows, tp:tp + 1],
                     False, TILE)
                nc.gpsimd.dma_start(
                    out_dram[128 * tp:128 * tp + rows, b0:b0 + TILE],
                    ot[0:rows, :])

            for k in range(NU + 4):
                if k < NU:
                    stage_l1(k)
                if 0 <= k - 2 < NU:
                    stage_l2(k - 2)
                if 0 <= k - 4 < NU:
                    stage_l3(k - 4)

    nc.compile()
    return nc


PACKED = None
_NC = None
LAST_RESULT = None


def prepare(inputs):
    """Build (once) the bass module and the per-core input maps."""
    global PACKED, _NC
    import sys
    if "/opt/trn_rl_repo" not in sys.path:
        sys.path.insert(0, "/opt/trn_rl_repo")
    x = np.asarray(inputs["x"], np.float32)
    PACKED = pack_weights(inputs)
    if _NC is None:
        _NC = build_bass_kernel()
    in_maps = []
    for core in range(NCORES):
        m = dict(PACKED)
        m["xpk"] = pack_x(x[core * BC:(core + 1) * BC])
        in_maps.append(m)
    return _NC, in_maps


def kernel(**inputs):
    global LAST_RESULT
    nc, in_maps = prepare(inputs)
    from concourse.bass_utils import run_bass_kernel_spmd
    res = run_bass_kernel_spmd(nc, in_maps, core_ids=list(range(NCORES)))
    LAST_RESULT = res
    # outf is [1344, BC] bf16 feature-major per core; unshard + transpose host-side.
    out = np.empty((B, J, D), np.float32)
    for core, r in enumerate(res.results):
        fm = np.asarray(r["outf"]).reshape(J, D, BC).astype(np.float32)
        out[core * BC:(core + 1) * BC] = fm.transpose(2, 0, 1)
    return out


# revision 14
# speedup vs baseline: 1.3264x; 1.0119x over previous
"""Trainium2 Bass kernel for the 21-joint hand-graph message-passing MLP.

Math (per sample b, per target joint t with neighbor list S_t of length n):
    g   = concat(x[b, S_t[0]], ..., x[b, S_t[n-1]])          # [n*64]
    h1  = relu(g @ W1_t + b1_t)                              # [128]
    h2  = relu(h1 @ W2_t + b2_t)                             # [128]
    out[b, t] = h2 @ W3_t + b3_t                             # [64]

Strategy (pure data parallel over 8 NeuronCores, B=65536 -> 8192/core):
  - x is pre-transposed ON THE HOST into a feature-major pair-tile layout
    xpk [15*128, BC] bf16: block p holds the 64 features of node u_p on
    partitions 0-63 and node v_p on 64-127.  The kernel reads it with ONE
    plain 3.84MB DMA per 1024-batch tile (sync HWDGE ring) -- no xbar
    transposes at all.  Host-side pair packing makes ARBITRARY node pairs
    available, so L1 runs the optimal per-target pairing: 28 genuine
    K=128 pair chunks + 13 zero-padded singles = 41 chunks (vs 47 for
    the adjacent-pair tile set), all full 128-row matmuls (partial-row
    matmuls that mix row groups in a PSUM bank can fault the device).
  - L2 is weight-stationary (w2 [128,128]); relu+bias fused into the PSUM
    evacuations, greedily load-balanced between ScalarE and VectorE.
  - L3 is w3-stationary with h2 moving at N=512: two adjacent targets
    share one 2-bank PSUM tile in disjoint col groups (concurrent
    matmuls); b3 is a per-partition bias folded into the single [128,1024]
    evacuation, which writes bf16.  Stores go out on the gpsimd SWDGE
    queues (separate from the load ring).  The feature-major [1344, BC]
    bf16 result is transposed back to [B, 21, 64] f32 on the host.
  - The PE stream is software-pipelined (L1(k) | L2(k-2) | L3(k-4)),
    continuous across batch-tile boundaries; the 2-step L2 lag hides the
    h1 PSUM-evacuation latency behind a full step of L1 matmuls.
"""

import numpy as np
import ml_dtypes

B, J, D, H1, H2 = 65536, 21, 64, 128, 128
NCORES = 8
BC = B // NCORES          # 8192 samples per core
TILE = 1024               # batch tile (psum1/psum2 = 2 PSUM banks in fp32)
NTILES = BC // TILE       # 8
TPAIRS = 11               # L3 target pairs: (0,1),(2,3),...,(18,19),(20,)

# Host-packed x pair tiles: ANY (u, v) node pair can be a tile because the
# host lays them out adjacently.  Chosen so every target's neighbor list
# splits into ceil(n/2) chunks (28 pairs + 13 singles = 41 chunks).
XTILES = [(0, 13), (1, 5), (9, 17), (0, 5), (1, 2), (3, 4), (5, 6),
          (0, 1), (7, 8), (9, 10), (11, 12), (13, 14), (15, 16),
          (17, 18), (19, 20)]
NPAIRS = len(XTILES)
PAIR_TILE = {p: i for i, p in enumerate(XTILES)}
TILE_OF = {}              # node -> (tile_idx, half), first occurrence
for _i, (_u, _v) in enumerate(XTILES):
    TILE_OF.setdefault(_u, (_i, 0))
    TILE_OF.setdefault(_v, (_i, 1))

FINGER_BASE = [4 * f + 1 for f in range(5)]
NEIGH = {
    6: [[0, 1, 5, 9, 13, 17]],
    5: [[0, 5, 6, 1, 9], [0, 9, 10, 5, 13], [0, 13, 14, 9, 17]],
    4: [[0, 1, 2, 5], [0, 17, 18, 13]],
    3: [r for b in FINGER_BASE for r in ([b, b + 1, b + 2], [b + 1, b + 2, b + 3])],
    2: [[b + 2, b + 3] for b in FINGER_BASE],
}
OUT = {
    6: [0],
    5: [5, 9, 13],
    4: [1, 17],
    3: [j for b in FINGER_BASE for j in (b + 1, b + 2)],
    2: [b + 3 for b in FINGER_BASE],
}
GROUPS = [6, 5, 4, 3, 2]

# target t -> (group n, row index within its group, neighbor list)
TARGET = {}
for n in GROUPS:
    for row, t in enumerate(OUT[n]):
        TARGET[t] = (n, row, list(NEIGH[n][row]))

# Hand-chosen optimal pairing per target (node pairs; all are in XTILES).
PAIRING = {
    0: [(0, 13), (1, 5), (9, 17)],
    1: [(0, 5), (1, 2)],
    2: [(1, 2)],
    3: [(3, 4)],
    4: [(3, 4)],
    5: [(5, 6), (0, 1)],
    6: [(5, 6)],
    7: [(7, 8)],
    8: [(7, 8)],
    9: [(9, 10), (0, 13)],
    10: [(9, 10)],
    11: [(11, 12)],
    12: [(11, 12)],
    13: [(13, 14), (9, 17)],
    14: [(13, 14)],
    15: [(15, 16)],
    16: [(15, 16)],
    17: [(17, 18), (0, 13)],
    18: [(17, 18)],
    19: [(19, 20)],
    20: [(19, 20)],
}


def build_l1_plan():
    plan = {}
    for t in range(21):
        n, _, S = TARGET[t]
        used = [False] * n
        pairs = []
        for (u, v) in PAIRING[t]:
            i, k = S.index(u), S.index(v)
            assert not used[i] and not used[k]
            used[i] = used[k] = True
            pairs.append(dict(tile=PAIR_TILE[(u, v)], pos0=i, pos1=k))
        singles = []
        for i in range(n):
            if not used[i]:
                tile_idx, half = TILE_OF[S[i]]
                singles.append(dict(tile=tile_idx, pos=i, half=half))
        assert len(singles) <= 1
        plan[t] = dict(pairs=pairs, singles=singles)
    return plan


L1_PLAN = build_l1_plan()


def assign_w1_cols():
    cols = {}
    col = 0
    for t in range(21):
        p = L1_PLAN[t]
        for i, _ in enumerate(p["pairs"]):
            cols[(t, "pair", i)] = col
            col += 128
        for i, _ in enumerate(p["singles"]):
            cols[(t, "single", i)] = col
            col += 128
    return cols, col


W1_COLS, W1_NCOL = assign_w1_cols()      # 41 * 128 = 5248


def pack_weights(inputs):
    """Host-side prep: permute/pack all weights into flat bf16/f32 arrays."""
    bf16 = ml_dtypes.bfloat16
    w1p = np.zeros((128, W1_NCOL), np.float32)
    for t in range(21):
        n, row, S = TARGET[t]
        W1 = np.asarray(inputs[f"w1_g{n}"][row], np.float32)  # [n*64, 128]
        p = L1_PLAN[t]
        for i, pr in enumerate(p["pairs"]):
            c = W1_COLS[(t, "pair", i)]
            w1p[0:64, c:c + 128] = W1[64 * pr["pos0"]:64 * pr["pos0"] + 64]
            w1p[64:128, c:c + 128] = W1[64 * pr["pos1"]:64 * pr["pos1"] + 64]
        for i, e in enumerate(p["singles"]):
            c = W1_COLS[(t, "single", i)]
            half = e["half"]
            w1p[64 * half:64 * half + 64, c:c + 128] = \
                W1[64 * e["pos"]:64 * e["pos"] + 64]
    w2p = np.zeros((128, 128 * 21), np.float32)
    w3p = np.zeros((128, 64 * 21), np.float32)
    b1p = np.zeros((128, 21), np.float32)
    b2p = np.zeros((128, 21), np.float32)
    b3p = np.zeros((128, TPAIRS), np.float32)   # per-partition bias, paired
    for t in range(21):
        n, row, _ = TARGET[t]
        w2p[:, 128 * t:128 * (t + 1)] = np.asarray(inputs[f"w2_g{n}"][row])
        w3p[:, 64 * t:64 * (t + 1)] = np.asarray(inputs[f"w3_g{n}"][row])
        b1p[:, t] = np.asarray(inputs[f"b1_g{n}"][row])
        b2p[:, t] = np.asarray(inputs[f"b2_g{n}"][row])
        b3p[64 * (t % 2):64 * (t % 2) + 64, t // 2] = \
            np.asarray(inputs[f"b3_g{n}"][row])
    return dict(
        w1p=w1p.astype(bf16), w2p=w2p.astype(bf16), w3p=w3p.astype(bf16),
        b1p=b1p, b2p=b2p, b3p=b3p,
    )


def pack_x(x):
    """x [Bn, 21, 64] f32 -> xpk [NPAIRS*128, Bn] bf16 (feature-major pairs)."""
    bf16 = ml_dtypes.bfloat16
    Bn = x.shape[0]
    xt = np.ascontiguousarray(
        np.asarray(x, np.float32).astype(bf16).transpose(1, 2, 0))  # [21,64,Bn]
    xpk = np.empty((NPAIRS * 128, Bn), bf16)
    for p, (u, v) in enumerate(XTILES):
        xpk[128 * p:128 * p + 64] = xt[u]
        xpk[128 * p + 64:128 * p + 128] = xt[v]
    return xpk


def numpy_emulate(inputs, x):
    """Bit-layout-faithful numpy model of the HW kernel (minus PSUM rounding):
    validates the chunk plan / packing / L3 pairing offline."""
    bf16 = ml_dtypes.bfloat16
    packed = pack_weights(inputs)
    xpk = pack_x(x)
    Bn = x.shape[0]
    out = np.zeros((Bn, 21, 64), np.float32)
    for t in range(21):
        psum1 = np.zeros((128, Bn), np.float32)
        pl = L1_PLAN[t]
        chunks = [("pair", i, pr["tile"]) for i, pr in enumerate(pl["pairs"])]
        chunks += [("single", i, e["tile"]) for i, e in enumerate(pl["singles"])]
        for kind, i, tl in chunks:
            c = W1_COLS[(t, kind, i)]
            lhsT = packed["w1p"][:, c:c + 128].astype(np.float32)
            rhs = xpk[128 * tl:128 * tl + 128].astype(np.float32)
            psum1 += lhsT.T @ rhs
        h1 = np.maximum(psum1 + packed["b1p"][:, t:t + 1], 0).astype(bf16)
        w2 = packed["w2p"][:, 128 * t:128 * (t + 1)].astype(np.float32)
        psum2 = w2.T @ h1.astype(np.float32)
        h2 = np.maximum(psum2 + packed["b2p"][:, t:t + 1], 0).astype(bf16)
        w3 = packed["w3p"][:, 64 * t:64 * (t + 1)].astype(np.float32)
        b3 = packed["b3p"][64 * (t % 2):64 * (t % 2) + 64, t // 2]
        o = (w3.T @ h2.astype(np.float32) + b3[:, None]).astype(bf16)
        out[:, t] = o.T.astype(np.float32)
    return out


# ---------------------------------------------------------------------------
# Bass kernel
# ---------------------------------------------------------------------------

def build_bass_kernel(num_devices=NCORES, bc=BC):
    import concourse.bass as bass
    import concourse.tile as tile
    from concourse import bacc, mybir

    bf16 = mybir.dt.bfloat16
    f32 = mybir.dt.float32
    Relu = mybir.ActivationFunctionType.Relu
    Ident = mybir.ActivationFunctionType.Identity
    Alu = mybir.AluOpType
    ntiles = bc // TILE

    nc = bacc.Bacc("TRN2", target_bir_lowering=False, debug=False,
                   num_devices=num_devices)
    x_dram = nc.dram_tensor("xpk", [NPAIRS * 128, bc], bf16,
                            kind="ExternalInput").ap()
    out_dram = nc.dram_tensor("outf", [J * D, bc], bf16,
                              kind="ExternalOutput").ap()
    w1_dram = nc.dram_tensor("w1p", [128, W1_NCOL], bf16, kind="ExternalInput").ap()
    w2_dram = nc.dram_tensor("w2p", [128, 128 * 21], bf16, kind="ExternalInput").ap()
    w3_dram = nc.dram_tensor("w3p", [128, 64 * 21], bf16, kind="ExternalInput").ap()
    b1_dram = nc.dram_tensor("b1p", [128, 21], f32, kind="ExternalInput").ap()
    b2_dram = nc.dram_tensor("b2p", [128, 21], f32, kind="ExternalInput").ap()
    b3_dram = nc.dram_tensor("b3p", [128, TPAIRS], f32, kind="ExternalInput").ap()
    # [128, NPAIRS, bc] view for the one-DMA-per-batch-tile slab load
    x_view = x_dram.rearrange("(p r) b -> r p b", p=NPAIRS, r=128)

    with tile.TileContext(nc) as tc:
        with (
            tc.tile_pool(name="wpool", bufs=1) as wpool,
            tc.tile_pool(name="xtp", bufs=3) as xtp,
            tc.tile_pool(name="h1p", bufs=4) as h1p,
            tc.tile_pool(name="h2p", bufs=1) as h2p,
            tc.tile_pool(name="outp", bufs=6) as outp,
            tc.tile_pool(name="psp", bufs=4, space="PSUM") as psp,
        ):
            w1s = wpool.tile([128, W1_NCOL], bf16, name="w1s")
            w2s = wpool.tile([128, 128 * 21], bf16, name="w2s")
            w3s = wpool.tile([128, 64 * 21], bf16, name="w3s")
            b1s = wpool.tile([128, 21], f32, name="b1s")
            b2s = wpool.tile([128, 21], f32, name="b2s")
            b3s = wpool.tile([128, TPAIRS], f32, name="b3s")
            dummy = wpool.tile([128, 640], bf16, name="dummy")

            # greedy ScalarE/VectorE balance for the PSUM evacuations,
            # using HW-measured op costs (ACT 1335ns / DVE 1536ns @ FD=1024)
            ev_time = [0.0, 0.0]          # ns on [ACT, DVE]

            def evac(dst, src, bias, relu, fd):
                act_cost = (312.0 + fd) / 1.2
                dve_cost = (210.0 + fd * 1.05) / 0.96
                if ev_time[0] + act_cost <= ev_time[1] + dve_cost:
                    ev_time[0] += act_cost
                    nc.scalar.activation(dst, src, Relu if relu else Ident,
                                         bias=bias, scale=1.0)
                else:
                    ev_time[1] += dve_cost
                    if relu:
                        nc.vector.tensor_scalar(dst, src, bias, 0.0,
                                                Alu.add, Alu.max)
                    else:
                        nc.vector.tensor_scalar(dst, src, bias, None, Alu.add)

            def mm2(out_ap, wcol_lo, wcol_hi, wtile, rhs_lo, rhs_hi,
                    start, stop, base=0):
                """One N=512 slot as two concurrent 64-col tile matmuls.
                All matmuls in the kernel use (128,64) tiling so the PE
                never switches tiling mode (mode switches drain the array
                and serialize; uniform mode lets T0/T1 run concurrently)."""
                nc.tensor.matmul(out_ap[base:base + 64, :], wtile[:, wcol_lo],
                                 rhs_lo, start=start, stop=stop,
                                 skip_group_check=True)
                nc.tensor.matmul(out_ap[base + 64:base + 128, :],
                                 wtile[:, wcol_hi],
                                 rhs_hi, start=start, stop=stop,
                                 skip_group_check=True)

            def issue_load(it, split=False):
                """ONE big DMA per batch-tile: a burst of small DMAs
                backlogs the HWDGE ring and the framework's DMA-semaphore
                reuse fences in the PE queue then stall on them.  The
                single DMA is issued two tiles ahead, so any fence on it
                is long-satisfied.  Tile 0 is split so the pair tiles the
                first targets need arrive first."""
                b0 = it * TILE
                slab = xtp.tile([128, NPAIRS * TILE], bf16, tag="slab",
                                name="slab")
                slab3 = slab[:].rearrange("r (p b) -> r p b", p=NPAIRS, b=TILE)
                if split:
                    for p0, p1 in ((0, 2), (2, 4), (4, 6), (6, 9), (9, 12),
                                   (12, NPAIRS)):
                        nc.sync.dma_start(slab3[:, p0:p1, :],
                                          x_view[:, p0:p1, b0:b0 + TILE])
                else:
                    nc.sync.dma_start(slab3, x_view[:, :, b0:b0 + TILE])
                return slab

            units = [(it, t) for it in range(ntiles) for t in range(21)]
            NU = len(units)
            # per-pair slab loads run TWO batch-tiles ahead on the sync
            # HWDGE ring, in first-use order; w1 is split so the columns
            # for the first targets arrive first.
            xts = {0: issue_load(0, split=True)}
            W1SPLIT = 2176       # end of target 7's w1 columns
            nc.scalar.dma_start(w1s[:, 0:W1SPLIT], w1_dram[:, 0:W1SPLIT])
            nc.scalar.dma_start(b1s[:], b1_dram)
            nc.scalar.dma_start(w2s[:], w2_dram)
            nc.scalar.dma_start(b2s[:], b2_dram)
            nc.scalar.dma_start(w3s[:], w3_dram)
            nc.scalar.dma_start(b3s[:], b3_dram)
            nc.scalar.dma_start(w1s[:, W1SPLIT:], w1_dram[:, W1SPLIT:])
            if ntiles > 1:
                xts[1] = issue_load(1)
            h1t = {}
            h2t = {}

            # PE warm-up: dummy (128,64)-tiled matmuls with no data deps
            # keep the PE busy through the initial load phase so the HAM
            # clock-gate is released before the first real matmul.
            nc.vector.memset(dummy[:], 0.0)
            warm = psp.tile([128, TILE], f32, tag="ps", name="ps")
            for _ in range(12):
                mm2(warm[:, 0:512], slice(0, 64), slice(64, 128), dummy,
                    dummy[:, 128:640], dummy[:, 128:640], True, True)

            def stage_l1(k):
                it, t = units[k]
                if t == 0 and it + 2 < ntiles:
                    xts[it + 2] = issue_load(it + 2)
                slab = xts[it]
                pl = L1_PLAN[t]
                psum1 = psp.tile([128, TILE], f32, tag="ps", name="ps")
                chunks = []
                for i, pr in enumerate(pl["pairs"]):
                    chunks.append((W1_COLS[(t, "pair", i)], pr["tile"]))
                for i, e in enumerate(pl["singles"]):
                    chunks.append((W1_COLS[(t, "single", i)], e["tile"]))
                nch = len(chunks)
                for ci, (c, tl) in enumerate(chunks):
                    for h in range(2):
                        rhs = slab[:, TILE * tl + 512 * h:TILE * tl + 512 * (h + 1)]
                        mm2(psum1[:, 512 * h:512 * (h + 1)],
                            slice(c, c + 64), slice(c + 64, c + 128), w1s,
                            rhs, rhs, ci == 0, ci == nch - 1)
                h1 = h1p.tile([128, TILE], bf16, tag="h1", name="h1")
                evac(h1[:], psum1[:], b1s[:, t:t + 1], True, TILE)
                h1t[k] = h1

            def stage_l2(k):
                it, t = units[k]
                h1 = h1t.pop(k)
                psum2 = psp.tile([128, TILE], f32, tag="ps", name="ps")
                for h in range(2):
                    rhs = h1[:, 512 * h:512 * (h + 1)]
                    mm2(psum2[:, 512 * h:512 * (h + 1)],
                        slice(128 * t, 128 * t + 64),
                        slice(128 * t + 64, 128 * (t + 1)), w2s,
                        rhs, rhs, True, True)
                h2 = h2p.tile([128, TILE], bf16, tag=f"h2_{t % 4}",
                              name=f"h2_{t % 4}")
                evac(h2[:], psum2[:], b2s[:, t:t + 1], True, TILE)
                h2t[k] = h2

            def stage_l3(k):
                it, t = units[k]
                if not (t % 2 == 1 or t == 20):
                    return
                b0 = it * TILE
                tp = t // 2
                if t % 2 == 1:
                    tlo, thi = t - 1, t
                    h2lo, h2hi = h2t.pop(k - 1), h2t.pop(k)
                    rows = 128
                else:
                    tlo, thi = t, None
                    h2lo, h2hi = h2t.pop(k), None
                    rows = 64
                ot = outp.tile([128, TILE], bf16, tag="ot", name="ot")
                psum3 = psp.tile([128, TILE], f32, tag="ps", name="ps")
                for h in range(2):
                    if thi is not None:
                        # tlo on col tile T0, thi on T1 — concurrent
                        mm2(psum3[:, 512 * h:512 * (h + 1)],
                            slice(64 * tlo, 64 * tlo + 64),
                            slice(64 * thi, 64 * thi + 64), w3s,
                            h2lo[:, 512 * h:512 * (h + 1)],
                            h2hi[:, 512 * h:512 * (h + 1)], True, True)
                    else:
                        nc.tensor.matmul(
                            psum3[0:64, 512 * h:512 * (h + 1)],
                            w3s[:, 64 * tlo:64 * tlo + 64],
                            h2lo[:, 512 * h:512 * (h + 1)],
                            start=True, stop=True, skip_group_check=True)
                evac(ot[0:rows, :], psum3[0:rows, :], b3s[0:rows, tp:tp + 1],
                     False, TILE)
                # final-tile stores go HWDGE (completes faster than the
                # gpsimd SWDGE path, shortening the end-of-kernel fence walk)
                dma = nc.scalar.dma_start if it == ntiles - 1 \
                    else nc.gpsimd.dma_start
                dma(out_dram[128 * tp:128 * tp + rows, b0:b0 + TILE],
                    ot[0:rows, :])

            for k in range(NU + 5):
                if k < NU:
                    stage_l1(k)
                if 0 <= k - 3 < NU:
                    stage_l2(k - 3)
                if 0 <= k - 5 < NU:
                    stage_l3(k - 5)

    nc.compile()
    return nc


PACKED = None
_NC = None
LAST_RESULT = None


def prepare(inputs):
    """Build (once) the bass module and the per-core input maps."""
    global PACKED, _NC
    import sys
    if "/opt/trn_rl_repo" not in sys.path:
        sys.path.insert(0, "/opt/trn_rl_repo")
    x = np.asarray(inputs["x"], np.float32)
    PACKED = pack_weights(inputs)
    if _NC is None:
        _NC = build_bass_kernel()
    in_maps = []
    for core in range(NCORES):
        m = dict(PACKED)
        m["xpk"] = pack_x(x[core * BC:(core + 1) * BC])
        in_maps.append(m)
    return _NC, in_maps


def kernel(**inputs):
    global LAST_RESULT
    nc, in_maps = prepare(inputs)
    from concourse.bass_utils import run_bass_kernel_spmd
    res = run_bass_kernel_spmd(nc, in_maps, core_ids=list(range(NCORES)))
    LAST_RESULT = res
    # outf is [1344, BC] bf16 feature-major per core; unshard + transpose host-side.
    out = np.empty((B, J, D), np.float32)
    for core, r in enumerate(res.results):
        fm = np.asarray(r["outf"]).reshape(J, D, BC).astype(np.float32)
        out[core * BC:(core + 1) * BC] = fm.transpose(2, 0, 1)
    return out
